# revision 14
# baseline (speedup 1.0000x reference)
"""Conv2d 3x3 via packed-K band matmuls, v3.

Mapping (per core, H-shard of 512 output rows):
  - 30-row output blocks, 18 per core (block 17 overlaps, only its last
    2 rows are kept). Moving operand: [K=128, N=512] where partition
    32*ci + j holds input row r0+j of channel ci (j in [0,32)).
  - Stationary per dx: ONE [128, 128] band covering all 4 output
    channels: entry (32ci+j, 30co+m) = k[co, ci, j-m, dx]; cols 120..127
    are zero padding (keeps NumWeights==128 so FWL kicks in). A single
    matmul per (dx, 512-col strip) produces all 4 co at PSUM partitions
    30co+m — 24 matmuls per block total, accumulating dx in PSUM.
  - DMA layouts are packed host-side so every transfer is a few MB with
    >=16KB contiguous per partition:
      xs:  [128, 18*4098] bf16  (blocks side by side, partition-major)
      ys:  [120, 18*4096] bf16  (partition 30co+m = out row r0(b)+m)
    Input lands in 5 chunked DMAs (2+4+4+4+4 blocks), output leaves in
    7 chunked DMAs (3,3,3,3,3,2,1 blocks). bf16 output halves write
    traffic; rel-err budget (2e-2) dwarfs the bf16 rounding (~2e-3).
  - PSUM: [128, 2048] f32 tiles (4 banks), 2 bufs; drained by one
    tensor_copy per half-block (2/3 DVE, 1/3 ACT) casting to bf16.
"""

import numpy as np

import concourse.bass as bass
import concourse.tile as tile
from concourse import bacc, mybir
from concourse.bass_utils import run_bass_kernel_spmd

N_CORES = 8
C = 4
H = 4096
W = 4096
SH = H // N_CORES          # 512 output rows per core
YB = 30                    # output rows per block
NBLK = 18                  # 17 regular + 1 overlapping tail block
WPAD = W + 2               # 4098
WO = W                     # 4096

IN_CHUNKS = [(0, 1), (1, 1)] + [(2 * k, 2) for k in range(1, 9)]
OUT_CHUNKS = [(b, 1) for b in range(NBLK)]

MM_DT = mybir.dt.bfloat16
F32 = mybir.dt.float32

_CACHE = {}


def _r0(b: int) -> int:
    return YB * b if b < NBLK - 1 else SH - YB  # block 17 overlaps: rows 482..512


def _build_program():
    nc = bacc.Bacc(
        "TRN2", target_bir_lowering=False, debug=False, num_devices=N_CORES
    )

    xs_d = nc.dram_tensor("xs", [128, NBLK * WPAD], MM_DT, kind="ExternalInput")
    bands_d = nc.dram_tensor("bands", [128, 3 * 128], MM_DT, kind="ExternalInput")
    ys_d = nc.dram_tensor("ys", [120, NBLK * WO], MM_DT, kind="ExternalOutput")

    xs = xs_d.ap()
    ys = ys_d.ap()

    in_starts = {b0: n for b0, n in IN_CHUNKS}
    out_starts = {b0: n for b0, n in OUT_CHUNKS}

    with tile.TileContext(nc) as tc:
        with (
            tc.tile_pool(name="bp", bufs=1) as bpool,
            tc.tile_pool(name="xp", bufs=3) as xpool,
            tc.tile_pool(name="op", bufs=5) as opool,
            tc.tile_pool(name="pp", bufs=2, space=bass.MemorySpace.PSUM) as ppool,
        ):
            bt = bpool.tile([128, 3 * 128], MM_DT, tag="bands", name="bt")
            nc.scalar.dma_start(out=bt[:], in_=bands_d.ap()[:])

            xt = ot = None
            in_b0 = out_b0 = out_n = 0
            whi = 0
            oci = 0
            for b in range(NBLK):
                if b in in_starts:
                    n = in_starts[b]
                    xt = xpool.tile([128, n * WPAD], MM_DT, tag="xt", name=f"xt{b}")
                    nc.sync.dma_start(
                        out=xt[:], in_=xs[:, b * WPAD : (b + n) * WPAD]
                    )
                    in_b0 = b
                if b in out_starts:
                    out_n = out_starts[b]
                    ot = opool.tile([128, out_n * WO], MM_DT, tag="ot", name=f"ot{b}")
                    out_b0 = b
                xoff = (b - in_b0) * WPAD
                ooff = (b - out_b0) * WO
                for wh in range(2):
                    ps = ppool.tile(
                        [128, 2048], F32, tag="ps", name=f"ps_{b}_{wh}"
                    )
                    for dx in range(3):
                        band = bt[:, 128 * dx : 128 * (dx + 1)]
                        for wc in range(4):
                            s = xoff + 2048 * wh + 512 * wc + dx
                            nc.tensor.matmul(
                                ps[:, 512 * wc : 512 * (wc + 1)],
                                band,
                                xt[:, s : s + 512],
                                start=(dx == 0),
                                stop=(dx == 2),
                                skip_group_check=True,
                            )
                    dst = ot[:, ooff + 2048 * wh : ooff + 2048 * wh + 2048]
                    if whi % 3 == 2:
                        nc.scalar.copy(dst, ps[:])
                    else:
                        nc.vector.tensor_copy(dst, ps[:])
                    whi += 1
                if b == out_b0 + out_n - 1:
                    eng = nc.sync
                    eng.dma_start(
                        out=ys[:, out_b0 * WO : (b + 1) * WO],
                        in_=ot[0:120, :],
                    )
                    oci += 1

    nc.compile()
    return nc


def _make_bands(kw: np.ndarray):
    import ml_dtypes

    bands = np.zeros((128, 3 * 128), dtype=np.float32)
    m = np.arange(YB)
    for dx in range(3):
        for co in range(C):
            for ci in range(C):
                for dy in range(3):
                    bands[32 * ci + m + dy, 128 * dx + 30 * co + m] = kw[
                        co, ci, dy, dx
                    ]
    return bands.astype(ml_dtypes.bfloat16)


def _prep_inputs(x: np.ndarray, kw: np.ndarray) -> list[dict]:
    import ml_dtypes

    bdt = ml_dtypes.bfloat16
    xpad = np.zeros((C, H + 2, W + 2), dtype=bdt)
    xpad[:, 1 : H + 1, 1 : W + 1] = x.astype(bdt)
    bands = _make_bands(kw)
    row_idx = np.empty((NBLK, 32), dtype=np.int64)
    for b in range(NBLK):
        row_idx[b] = _r0(b) + np.arange(32)
    maps = []
    for c in range(N_CORES):
        g = xpad[:, SH * c + row_idx, :]            # [C, NBLK, 32, WPAD]
        packed = np.ascontiguousarray(
            g.transpose(0, 2, 1, 3).reshape(128, NBLK * WPAD)
        )
        maps.append({"xs": packed, "bands": bands})
    return maps


def _unshard(results) -> np.ndarray:
    out = np.empty((C, H, W), dtype=np.float32)
    for c in range(N_CORES):
        ys_c = np.asarray(results[c]["ys"])          # [120, NBLK*WO] bf16
        t = ys_c.reshape(C, YB, NBLK, WO).transpose(0, 2, 1, 3)  # [co, b, m, w]
        oc = out[:, SH * c : SH * (c + 1), :]
        oc[:, : YB * (NBLK - 1)] = t[:, : NBLK - 1].reshape(C, YB * (NBLK - 1), WO)
        oc[:, YB * (NBLK - 1) :] = t[:, NBLK - 1, YB - 2 :]
    return out


def kernel(x: np.ndarray, kernel: np.ndarray) -> np.ndarray:
    x = np.asarray(x, dtype=np.float32)
    kw = np.asarray(kernel, dtype=np.float32)

    if "nc" not in _CACHE:
        _CACHE["nc"] = _build_program()
    nc = _CACHE["nc"]

    in_maps = _prep_inputs(x, kw)
    res = run_bass_kernel_spmd(nc, in_maps, list(range(N_CORES)))
    return _unshard(res.results)


# revision 18
# speedup vs baseline: 1.2954x; 1.2954x over previous
"""Conv2d 3x3 via packed-K band matmuls, v3.

Mapping (per core, H-shard of 512 output rows):
  - 30-row output blocks, 18 per core (block 17 overlaps, only its last
    2 rows are kept). Moving operand: [K=128, N=512] where partition
    32*ci + j holds input row r0+j of channel ci (j in [0,32)).
  - Stationary per dx: ONE [128, 128] band covering all 4 output
    channels: entry (32ci+j, 30co+m) = k[co, ci, j-m, dx]; cols 120..127
    are zero padding (keeps NumWeights==128 so FWL kicks in). A single
    matmul per (dx, 512-col strip) produces all 4 co at PSUM partitions
    30co+m — 24 matmuls per block total, accumulating dx in PSUM.
  - DMA layouts are packed host-side so every transfer is a few MB with
    >=16KB contiguous per partition:
      xs:  [128, 18*4098] bf16  (blocks side by side, partition-major)
      ys:  [120, 18*4096] bf16  (partition 30co+m = out row r0(b)+m)
    Input lands in 5 chunked DMAs (2+4+4+4+4 blocks), output leaves in
    7 chunked DMAs (3,3,3,3,3,2,1 blocks). bf16 output halves write
    traffic; rel-err budget (2e-2) dwarfs the bf16 rounding (~2e-3).
  - PSUM: [128, 2048] f32 tiles (4 banks), 2 bufs; drained by one
    tensor_copy per half-block (2/3 DVE, 1/3 ACT) casting to bf16.
"""

import numpy as np

import concourse.bass as bass
import concourse.tile as tile
from concourse import bacc, mybir
from concourse.bass_utils import run_bass_kernel_spmd

N_CORES = 8
C = 4
H = 4096
W = 4096
SH = H // N_CORES          # 512 output rows per core
YB = 30                    # output rows per block
NBLK = 18                  # 17 regular + 1 overlapping tail block
WPAD = W + 2               # 4098
WO = W                     # 4096

IN_CHUNKS = [(0, 1), (1, 1)] + [(2 * k, 2) for k in range(1, 9)]
OUT_CHUNKS = [(b, 1) for b in range(NBLK)]

MM_DT = mybir.dt.bfloat16
F32 = mybir.dt.float32

_CACHE = {}


def _r0(b: int) -> int:
    return YB * b if b < NBLK - 1 else SH - YB  # block 17 overlaps: rows 482..512


def _build_program():
    nc = bacc.Bacc(
        "TRN2", target_bir_lowering=False, debug=False, num_devices=N_CORES
    )

    xs_d = nc.dram_tensor("xs", [128, NBLK * WPAD], MM_DT, kind="ExternalInput")
    bands_d = nc.dram_tensor("bands", [128, 3 * 128], MM_DT, kind="ExternalInput")
    ys_d = nc.dram_tensor("ys", [120, NBLK * WO], MM_DT, kind="ExternalOutput")

    xs = xs_d.ap()
    ys = ys_d.ap()

    in_starts = {b0: n for b0, n in IN_CHUNKS}
    out_starts = {b0: n for b0, n in OUT_CHUNKS}

    with tile.TileContext(nc) as tc:
        with (
            tc.tile_pool(name="bp", bufs=1) as bpool,
            tc.tile_pool(name="xp", bufs=3) as xpool,
            tc.tile_pool(name="op", bufs=5) as opool,
            tc.tile_pool(name="pp", bufs=4, space=bass.MemorySpace.PSUM) as ppool,
        ):
            bt = bpool.tile([128, 3 * 128], MM_DT, tag="bands", name="bt")
            nc.scalar.dma_start(out=bt[:], in_=bands_d.ap()[:])

            xt = ot = None
            in_b0 = out_b0 = out_n = 0
            whi = 0
            oci = 0
            for b in range(NBLK):
                if b in in_starts:
                    n = in_starts[b]
                    xt = xpool.tile([128, n * WPAD], MM_DT, tag="xt", name=f"xt{b}")
                    nc.sync.dma_start(
                        out=xt[:], in_=xs[:, b * WPAD : (b + n) * WPAD]
                    )
                    in_b0 = b
                if b in out_starts:
                    out_n = out_starts[b]
                    ot = opool.tile([128, out_n * WO], MM_DT, tag="ot", name=f"ot{b}")
                    out_b0 = b
                xoff = (b - in_b0) * WPAD
                ooff = (b - out_b0) * WO
                for u in range(4):
                    ps = ppool.tile(
                        [128, 1024], F32, tag="ps", name=f"ps_{b}_{u}"
                    )
                    for dx in range(3):
                        band = bt[:, 128 * dx : 128 * (dx + 1)]
                        for wc in range(2):
                            s = xoff + 1024 * u + 512 * wc + dx
                            nc.tensor.matmul(
                                ps[:, 512 * wc : 512 * (wc + 1)],
                                band,
                                xt[:, s : s + 512],
                                start=(dx == 0),
                                stop=(dx == 2),
                                skip_group_check=True,
                            )
                    dst = ot[:, ooff + 1024 * u : ooff + 1024 * u + 1024]
                    if whi % 4 == 3:
                        nc.scalar.copy(dst, ps[:])
                    else:
                        nc.vector.tensor_copy(dst, ps[:])
                    whi += 1
                if b == out_b0 + out_n - 1:
                    eng = nc.sync if oci % 2 == 0 else nc.scalar
                    eng.dma_start(
                        out=ys[:, out_b0 * WO : (b + 1) * WO],
                        in_=ot[0:120, :],
                    )
                    oci += 1

    nc.compile()
    return nc


def _make_bands(kw: np.ndarray):
    import ml_dtypes

    bands = np.zeros((128, 3 * 128), dtype=np.float32)
    m = np.arange(YB)
    for dx in range(3):
        for co in range(C):
            for ci in range(C):
                for dy in range(3):
                    bands[32 * ci + m + dy, 128 * dx + 30 * co + m] = kw[
                        co, ci, dy, dx
                    ]
    return bands.astype(ml_dtypes.bfloat16)


def _prep_inputs(x: np.ndarray, kw: np.ndarray) -> list[dict]:
    import ml_dtypes

    bdt = ml_dtypes.bfloat16
    xpad = np.zeros((C, H + 2, W + 2), dtype=bdt)
    xpad[:, 1 : H + 1, 1 : W + 1] = x.astype(bdt)
    bands = _make_bands(kw)
    row_idx = np.empty((NBLK, 32), dtype=np.int64)
    for b in range(NBLK):
        row_idx[b] = _r0(b) + np.arange(32)
    maps = []
    for c in range(N_CORES):
        g = xpad[:, SH * c + row_idx, :]            # [C, NBLK, 32, WPAD]
        packed = np.ascontiguousarray(
            g.transpose(0, 2, 1, 3).reshape(128, NBLK * WPAD)
        )
        maps.append({"xs": packed, "bands": bands})
    return maps


def _unshard(results) -> np.ndarray:
    out = np.empty((C, H, W), dtype=np.float32)
    for c in range(N_CORES):
        ys_c = np.asarray(results[c]["ys"])          # [120, NBLK*WO] bf16
        t = ys_c.reshape(C, YB, NBLK, WO).transpose(0, 2, 1, 3)  # [co, b, m, w]
        oc = out[:, SH * c : SH * (c + 1), :]
        oc[:, : YB * (NBLK - 1)] = t[:, : NBLK - 1].reshape(C, YB * (NBLK - 1), WO)
        oc[:, YB * (NBLK - 1) :] = t[:, NBLK - 1, YB - 2 :]
    return out


def kernel(x: np.ndarray, kernel: np.ndarray) -> np.ndarray:
    x = np.asarray(x, dtype=np.float32)
    kw = np.asarray(kernel, dtype=np.float32)

    if "nc" not in _CACHE:
        _CACHE["nc"] = _build_program()
    nc = _CACHE["nc"]

    in_maps = _prep_inputs(x, kw)
    res = run_bass_kernel_spmd(nc, in_maps, list(range(N_CORES)))
    return _unshard(res.results)


# revision 20
# speedup vs baseline: 1.2989x; 1.0027x over previous
"""Conv2d 3x3 via packed-K band matmuls, v3.

Mapping (per core, H-shard of 512 output rows):
  - 30-row output blocks, 18 per core (block 17 overlaps, only its last
    2 rows are kept). Moving operand: [K=128, N=512] where partition
    32*ci + j holds input row r0+j of channel ci (j in [0,32)).
  - Stationary per dx: ONE [128, 128] band covering all 4 output
    channels: entry (32ci+j, 30co+m) = k[co, ci, j-m, dx]; cols 120..127
    are zero padding (keeps NumWeights==128 so FWL kicks in). A single
    matmul per (dx, 512-col strip) produces all 4 co at PSUM partitions
    30co+m — 24 matmuls per block total, accumulating dx in PSUM.
  - DMA layouts are packed host-side so every transfer is a few MB with
    >=16KB contiguous per partition:
      xs:  [128, 18*4098] bf16  (blocks side by side, partition-major)
      ys:  [120, 18*4096] bf16  (partition 30co+m = out row r0(b)+m)
    Input lands in 5 chunked DMAs (2+4+4+4+4 blocks), output leaves in
    7 chunked DMAs (3,3,3,3,3,2,1 blocks). bf16 output halves write
    traffic; rel-err budget (2e-2) dwarfs the bf16 rounding (~2e-3).
  - PSUM: [128, 2048] f32 tiles (4 banks), 2 bufs; drained by one
    tensor_copy per half-block (2/3 DVE, 1/3 ACT) casting to bf16.
"""

import numpy as np

import concourse.bass as bass
import concourse.tile as tile
from concourse import bacc, mybir
from concourse.bass_utils import run_bass_kernel_spmd

N_CORES = 8
C = 4
H = 4096
W = 4096
SH = H // N_CORES          # 512 output rows per core
YB = 30                    # output rows per block
NBLK = 18                  # 17 regular + 1 overlapping tail block
WPAD = W + 2               # 4098
WO = W                     # 4096

IN_CHUNKS = [(0, 1), (1, 1)] + [(2 * k, 2) for k in range(1, 9)]
OUT_CHUNKS = [(b, 1) for b in range(NBLK)]

MM_DT = mybir.dt.bfloat16
F32 = mybir.dt.float32

_CACHE = {}


def _r0(b: int) -> int:
    return YB * b if b < NBLK - 1 else SH - YB  # block 17 overlaps: rows 482..512


def _build_program():
    nc = bacc.Bacc(
        "TRN2", target_bir_lowering=False, debug=False, num_devices=N_CORES
    )

    xs_d = nc.dram_tensor("xs", [128, NBLK * WPAD], MM_DT, kind="ExternalInput")
    bands_d = nc.dram_tensor("bands", [128, 3 * 128], MM_DT, kind="ExternalInput")
    ys_d = nc.dram_tensor("ys", [120, NBLK * WO], MM_DT, kind="ExternalOutput")

    xs = xs_d.ap()
    ys = ys_d.ap()

    in_starts = {b0: n for b0, n in IN_CHUNKS}
    out_starts = {b0: n for b0, n in OUT_CHUNKS}

    with tile.TileContext(nc) as tc:
        with (
            tc.tile_pool(name="bp", bufs=1) as bpool,
            tc.tile_pool(name="xp", bufs=3) as xpool,
            tc.tile_pool(name="op", bufs=5) as opool,
            tc.tile_pool(name="pp", bufs=4, space=bass.MemorySpace.PSUM) as ppool,
        ):
            bt = bpool.tile([128, 3 * 128], MM_DT, tag="bands", name="bt")
            nc.scalar.dma_start(out=bt[:], in_=bands_d.ap()[:])

            xt = ot = None
            in_b0 = out_b0 = out_n = 0
            whi = 0
            oci = 0
            for b in range(NBLK):
                if b in in_starts:
                    n = in_starts[b]
                    xt = xpool.tile([128, n * WPAD], MM_DT, tag="xt", name=f"xt{b}")
                    if b == 0:
                        # split so the first matmuls start after half the load
                        nc.sync.dma_start(out=xt[:, :2052], in_=xs[:, :2052])
                        nc.sync.dma_start(
                            out=xt[:, 2052:WPAD], in_=xs[:, 2052:WPAD]
                        )
                    else:
                        nc.sync.dma_start(
                            out=xt[:], in_=xs[:, b * WPAD : (b + n) * WPAD]
                        )
                    in_b0 = b
                if b in out_starts:
                    out_n = out_starts[b]
                    ot = opool.tile([128, out_n * WO], MM_DT, tag="ot", name=f"ot{b}")
                    out_b0 = b
                xoff = (b - in_b0) * WPAD
                ooff = (b - out_b0) * WO
                for u in range(4):
                    ps = ppool.tile(
                        [128, 1024], F32, tag="ps", name=f"ps_{b}_{u}"
                    )
                    for dx in range(3):
                        band = bt[:, 128 * dx : 128 * (dx + 1)]
                        for wc in range(2):
                            s = xoff + 1024 * u + 512 * wc + dx
                            nc.tensor.matmul(
                                ps[:, 512 * wc : 512 * (wc + 1)],
                                band,
                                xt[:, s : s + 512],
                                start=(dx == 0),
                                stop=(dx == 2),
                                skip_group_check=True,
                            )
                    dst = ot[:, ooff + 1024 * u : ooff + 1024 * u + 1024]
                    if whi % 4 == 3:
                        nc.scalar.copy(dst, ps[:])
                    else:
                        nc.vector.tensor_copy(dst, ps[:])
                    whi += 1
                    if b == NBLK - 1 and u in (1, 3):
                        # tail: overlap the last write with the last copies
                        eng = nc.sync if u == 1 else nc.scalar
                        eng.dma_start(
                            out=ys[:, b * WO + (u - 1) * 1024 : b * WO + (u + 1) * 1024],
                            in_=ot[0:120, ooff + (u - 1) * 1024 : ooff + (u + 1) * 1024],
                        )
                if b == out_b0 + out_n - 1 and b != NBLK - 1:
                    eng = nc.sync if oci % 2 == 0 else nc.scalar
                    eng.dma_start(
                        out=ys[:, out_b0 * WO : (b + 1) * WO],
                        in_=ot[0:120, :],
                    )
                    oci += 1

    nc.compile()
    return nc


def _make_bands(kw: np.ndarray):
    import ml_dtypes

    bands = np.zeros((128, 3 * 128), dtype=np.float32)
    m = np.arange(YB)
    for dx in range(3):
        for co in range(C):
            for ci in range(C):
                for dy in range(3):
                    bands[32 * ci + m + dy, 128 * dx + 30 * co + m] = kw[
                        co, ci, dy, dx
                    ]
    return bands.astype(ml_dtypes.bfloat16)


def _prep_inputs(x: np.ndarray, kw: np.ndarray) -> list[dict]:
    import ml_dtypes

    bdt = ml_dtypes.bfloat16
    xpad = np.zeros((C, H + 2, W + 2), dtype=bdt)
    xpad[:, 1 : H + 1, 1 : W + 1] = x.astype(bdt)
    bands = _make_bands(kw)
    row_idx = np.empty((NBLK, 32), dtype=np.int64)
    for b in range(NBLK):
        row_idx[b] = _r0(b) + np.arange(32)
    maps = []
    for c in range(N_CORES):
        g = xpad[:, SH * c + row_idx, :]            # [C, NBLK, 32, WPAD]
        packed = np.ascontiguousarray(
            g.transpose(0, 2, 1, 3).reshape(128, NBLK * WPAD)
        )
        maps.append({"xs": packed, "bands": bands})
    return maps


def _unshard(results) -> np.ndarray:
    out = np.empty((C, H, W), dtype=np.float32)
    for c in range(N_CORES):
        ys_c = np.asarray(results[c]["ys"])          # [120, NBLK*WO] bf16
        t = ys_c.reshape(C, YB, NBLK, WO).transpose(0, 2, 1, 3)  # [co, b, m, w]
        oc = out[:, SH * c : SH * (c + 1), :]
        oc[:, : YB * (NBLK - 1)] = t[:, : NBLK - 1].reshape(C, YB * (NBLK - 1), WO)
        oc[:, YB * (NBLK - 1) :] = t[:, NBLK - 1, YB - 2 :]
    return out


def kernel(x: np.ndarray, kernel: np.ndarray) -> np.ndarray:
    x = np.asarray(x, dtype=np.float32)
    kw = np.asarray(kernel, dtype=np.float32)

    if "nc" not in _CACHE:
        _CACHE["nc"] = _build_program()
    nc = _CACHE["nc"]

    in_maps = _prep_inputs(x, kw)
    res = run_bass_kernel_spmd(nc, in_maps, list(range(N_CORES)))
    return _unshard(res.results)


# revision 21
# speedup vs baseline: 1.3069x; 1.0062x over previous
"""Conv2d 3x3 via packed-K band matmuls, v3.

Mapping (per core, H-shard of 512 output rows):
  - 30-row output blocks, 18 per core (block 17 overlaps, only its last
    2 rows are kept). Moving operand: [K=128, N=512] where partition
    32*ci + j holds input row r0+j of channel ci (j in [0,32)).
  - Stationary per dx: ONE [128, 128] band covering all 4 output
    channels: entry (32ci+j, 30co+m) = k[co, ci, j-m, dx]; cols 120..127
    are zero padding (keeps NumWeights==128 so FWL kicks in). A single
    matmul per (dx, 512-col strip) produces all 4 co at PSUM partitions
    30co+m — 24 matmuls per block total, accumulating dx in PSUM.
  - DMA layouts are packed host-side so every transfer is a few MB with
    >=16KB contiguous per partition:
      xs:  [128, 18*4098] bf16  (blocks side by side, partition-major)
      ys:  [120, 18*4096] bf16  (partition 30co+m = out row r0(b)+m)
    Input lands in 5 chunked DMAs (2+4+4+4+4 blocks), output leaves in
    7 chunked DMAs (3,3,3,3,3,2,1 blocks). bf16 output halves write
    traffic; rel-err budget (2e-2) dwarfs the bf16 rounding (~2e-3).
  - PSUM: [128, 2048] f32 tiles (4 banks), 2 bufs; drained by one
    tensor_copy per half-block (2/3 DVE, 1/3 ACT) casting to bf16.
"""

import numpy as np

import concourse.bass as bass
import concourse.tile as tile
from concourse import bacc, mybir
from concourse.bass_utils import run_bass_kernel_spmd

N_CORES = 8
C = 4
H = 4096
W = 4096
SH = H // N_CORES          # 512 output rows per core
YB = 30                    # output rows per block
NBLK = 18                  # 17 regular + 1 overlapping tail block
WPAD = W + 2               # 4098
WO = W                     # 4096

IN_CHUNKS = [(0, 1), (1, 1)] + [(2 * k, 2) for k in range(1, 9)]
OUT_CHUNKS = [(b, 1) for b in range(NBLK)]

MM_DT = mybir.dt.bfloat16
F32 = mybir.dt.float32

_CACHE = {}


def _r0(b: int) -> int:
    return YB * b if b < NBLK - 1 else SH - YB  # block 17 overlaps: rows 482..512


def _build_program():
    nc = bacc.Bacc(
        "TRN2", target_bir_lowering=False, debug=False, num_devices=N_CORES
    )

    xs_d = nc.dram_tensor("xs", [128, NBLK * WPAD], MM_DT, kind="ExternalInput")
    bands_d = nc.dram_tensor("bands", [128, 3 * 128], MM_DT, kind="ExternalInput")
    ys_d = nc.dram_tensor("ys", [120, NBLK * WO], MM_DT, kind="ExternalOutput")

    xs = xs_d.ap()
    ys = ys_d.ap()

    in_starts = {b0: n for b0, n in IN_CHUNKS}
    out_starts = {b0: n for b0, n in OUT_CHUNKS}

    with tile.TileContext(nc) as tc:
        with (
            tc.tile_pool(name="bp", bufs=1) as bpool,
            tc.tile_pool(name="xp", bufs=3) as xpool,
            tc.tile_pool(name="op", bufs=5) as opool,
            tc.tile_pool(name="pp", bufs=4, space=bass.MemorySpace.PSUM) as ppool,
        ):
            bt = bpool.tile([128, 3 * 128], MM_DT, tag="bands", name="bt")
            nc.scalar.dma_start(out=bt[:], in_=bands_d.ap()[:])

            # PE warmup: keep the HAM activity window busy during the DMA
            # head so the real matmul stream starts at full clock.
            wt = bpool.tile([128, 512], MM_DT, tag="warm", name="wt")
            nc.vector.memset(wt[:], 0)
            wps = ppool.tile([128, 512], F32, tag="ps", name="wps")
            for _ in range(10):
                nc.tensor.matmul(
                    wps[:, :512],
                    wt[:, :128],
                    wt[:, :512],
                    start=True,
                    stop=True,
                    skip_group_check=True,
                )

            xt = ot = None
            in_b0 = out_b0 = out_n = 0
            whi = 0
            oci = 0
            for b in range(NBLK):
                if b in in_starts:
                    n = in_starts[b]
                    xt = xpool.tile([128, n * WPAD], MM_DT, tag="xt", name=f"xt{b}")
                    if b == 0:
                        # split so the first matmuls start after half the load
                        nc.sync.dma_start(out=xt[:, :2052], in_=xs[:, :2052])
                        nc.sync.dma_start(
                            out=xt[:, 2052:WPAD], in_=xs[:, 2052:WPAD]
                        )
                    else:
                        nc.sync.dma_start(
                            out=xt[:], in_=xs[:, b * WPAD : (b + n) * WPAD]
                        )
                    in_b0 = b
                if b in out_starts:
                    out_n = out_starts[b]
                    ot = opool.tile([128, out_n * WO], MM_DT, tag="ot", name=f"ot{b}")
                    out_b0 = b
                xoff = (b - in_b0) * WPAD
                ooff = (b - out_b0) * WO
                for u in range(4):
                    ps = ppool.tile(
                        [128, 1024], F32, tag="ps", name=f"ps_{b}_{u}"
                    )
                    for dx in range(3):
                        band = bt[:, 128 * dx : 128 * (dx + 1)]
                        for wc in range(2):
                            s = xoff + 1024 * u + 512 * wc + dx
                            nc.tensor.matmul(
                                ps[:, 512 * wc : 512 * (wc + 1)],
                                band,
                                xt[:, s : s + 512],
                                start=(dx == 0),
                                stop=(dx == 2),
                                skip_group_check=True,
                            )
                    dst = ot[:, ooff + 1024 * u : ooff + 1024 * u + 1024]
                    if whi % 4 == 3:
                        nc.scalar.copy(dst, ps[:])
                    else:
                        nc.vector.tensor_copy(dst, ps[:])
                    whi += 1
                    if b == NBLK - 1 and u in (1, 3):
                        # tail: overlap the last write with the last copies
                        eng = nc.sync if u == 1 else nc.scalar
                        eng.dma_start(
                            out=ys[:, b * WO + (u - 1) * 1024 : b * WO + (u + 1) * 1024],
                            in_=ot[0:120, ooff + (u - 1) * 1024 : ooff + (u + 1) * 1024],
                        )
                if b == out_b0 + out_n - 1 and b != NBLK - 1:
                    eng = nc.sync if oci % 2 == 0 else nc.scalar
                    eng.dma_start(
                        out=ys[:, out_b0 * WO : (b + 1) * WO],
                        in_=ot[0:120, :],
                    )
                    oci += 1

    nc.compile()
    return nc


def _make_bands(kw: np.ndarray):
    import ml_dtypes

    bands = np.zeros((128, 3 * 128), dtype=np.float32)
    m = np.arange(YB)
    for dx in range(3):
        for co in range(C):
            for ci in range(C):
                for dy in range(3):
                    bands[32 * ci + m + dy, 128 * dx + 30 * co + m] = kw[
                        co, ci, dy, dx
                    ]
    return bands.astype(ml_dtypes.bfloat16)


def _prep_inputs(x: np.ndarray, kw: np.ndarray) -> list[dict]:
    import ml_dtypes

    bdt = ml_dtypes.bfloat16
    xpad = np.zeros((C, H + 2, W + 2), dtype=bdt)
    xpad[:, 1 : H + 1, 1 : W + 1] = x.astype(bdt)
    bands = _make_bands(kw)
    row_idx = np.empty((NBLK, 32), dtype=np.int64)
    for b in range(NBLK):
        row_idx[b] = _r0(b) + np.arange(32)
    maps = []
    for c in range(N_CORES):
        g = xpad[:, SH * c + row_idx, :]            # [C, NBLK, 32, WPAD]
        packed = np.ascontiguousarray(
            g.transpose(0, 2, 1, 3).reshape(128, NBLK * WPAD)
        )
        maps.append({"xs": packed, "bands": bands})
    return maps


def _unshard(results) -> np.ndarray:
    out = np.empty((C, H, W), dtype=np.float32)
    for c in range(N_CORES):
        ys_c = np.asarray(results[c]["ys"])          # [120, NBLK*WO] bf16
        t = ys_c.reshape(C, YB, NBLK, WO).transpose(0, 2, 1, 3)  # [co, b, m, w]
        oc = out[:, SH * c : SH * (c + 1), :]
        oc[:, : YB * (NBLK - 1)] = t[:, : NBLK - 1].reshape(C, YB * (NBLK - 1), WO)
        oc[:, YB * (NBLK - 1) :] = t[:, NBLK - 1, YB - 2 :]
    return out


def kernel(x: np.ndarray, kernel: np.ndarray) -> np.ndarray:
    x = np.asarray(x, dtype=np.float32)
    kw = np.asarray(kernel, dtype=np.float32)

    if "nc" not in _CACHE:
        _CACHE["nc"] = _build_program()
    nc = _CACHE["nc"]

    in_maps = _prep_inputs(x, kw)
    res = run_bass_kernel_spmd(nc, in_maps, list(range(N_CORES)))
    return _unshard(res.results)


# revision 23
# speedup vs baseline: 1.3332x; 1.0201x over previous
"""Conv2d 3x3 via packed-K band matmuls, v3.

Mapping (per core, H-shard of 512 output rows):
  - 30-row output blocks, 18 per core (block 17 overlaps, only its last
    2 rows are kept). Moving operand: [K=128, N=512] where partition
    32*ci + j holds input row r0+j of channel ci (j in [0,32)).
  - Stationary per dx: ONE [128, 128] band covering all 4 output
    channels: entry (32ci+j, 30co+m) = k[co, ci, j-m, dx]; cols 120..127
    are zero padding (keeps NumWeights==128 so FWL kicks in). A single
    matmul per (dx, 512-col strip) produces all 4 co at PSUM partitions
    30co+m — 24 matmuls per block total, accumulating dx in PSUM.
  - DMA layouts are packed host-side so every transfer is a few MB with
    >=16KB contiguous per partition:
      xs:  [128, 18*4098] bf16  (blocks side by side, partition-major)
      ys:  [120, 18*4096] bf16  (partition 30co+m = out row r0(b)+m)
    Input lands in 5 chunked DMAs (2+4+4+4+4 blocks), output leaves in
    7 chunked DMAs (3,3,3,3,3,2,1 blocks). bf16 output halves write
    traffic; rel-err budget (2e-2) dwarfs the bf16 rounding (~2e-3).
  - PSUM: [128, 2048] f32 tiles (4 banks), 2 bufs; drained by one
    tensor_copy per half-block (2/3 DVE, 1/3 ACT) casting to bf16.
"""

import numpy as np

import concourse.bass as bass
import concourse.tile as tile
from concourse import bacc, mybir
from concourse.bass_utils import run_bass_kernel_spmd

N_CORES = 8
C = 4
H = 4096
W = 4096
SH = H // N_CORES          # 512 output rows per core
YB = 30                    # output rows per block
NBLK = 18                  # 17 regular + 1 overlapping tail block
WPAD = W + 2               # 4098
WO = W                     # 4096

IN_CHUNKS = [(0, 1), (1, 1)] + [(2 * k, 2) for k in range(1, 9)]
OUT_CHUNKS = [(b, 1) for b in range(NBLK)]

MM_DT = mybir.dt.bfloat16
F32 = mybir.dt.float32

_CACHE = {}


def _r0(b: int) -> int:
    return YB * b if b < NBLK - 1 else SH - YB  # block 17 overlaps: rows 482..512


def _build_program():
    nc = bacc.Bacc(
        "TRN2", target_bir_lowering=False, debug=False, num_devices=N_CORES
    )

    xs_d = nc.dram_tensor("xs", [128, NBLK * WPAD], MM_DT, kind="ExternalInput")
    bands_d = nc.dram_tensor("bands", [128, 3 * 128], MM_DT, kind="ExternalInput")
    ys_d = nc.dram_tensor("ys", [120, NBLK * WO], MM_DT, kind="ExternalOutput")

    xs = xs_d.ap()
    ys = ys_d.ap()

    in_starts = {b0: n for b0, n in IN_CHUNKS}
    out_starts = {b0: n for b0, n in OUT_CHUNKS}

    with tile.TileContext(nc) as tc:
        with (
            tc.tile_pool(name="bp", bufs=1) as bpool,
            tc.tile_pool(name="xp", bufs=3) as xpool,
            tc.tile_pool(name="op", bufs=5) as opool,
            tc.tile_pool(name="pp", bufs=8, space=bass.MemorySpace.PSUM) as ppool,
        ):
            bt = bpool.tile([128, 3 * 128], MM_DT, tag="bands", name="bt")
            nc.scalar.dma_start(out=bt[:], in_=bands_d.ap()[:])

            # PE warmup: keep the HAM activity window busy during the DMA
            # head so the real matmul stream starts at full clock.
            wt = bpool.tile([128, 512], MM_DT, tag="warm", name="wt")
            nc.vector.memset(wt[:], 0)
            wps = ppool.tile([128, 512], F32, tag="ps", name="wps")
            for _ in range(10):
                nc.tensor.matmul(
                    wps[:, :512],
                    wt[:, :128],
                    wt[:, :512],
                    start=True,
                    stop=True,
                    skip_group_check=True,
                )

            xt = ot = None
            in_b0 = out_b0 = out_n = 0
            whi = 0
            oci = 0
            for b in range(NBLK):
                if b in in_starts:
                    n = in_starts[b]
                    xt = xpool.tile([128, n * WPAD], MM_DT, tag="xt", name=f"xt{b}")
                    if b == 0:
                        # split so the first matmuls start after half the load
                        nc.sync.dma_start(out=xt[:, :2052], in_=xs[:, :2052])
                        nc.sync.dma_start(
                            out=xt[:, 2052:WPAD], in_=xs[:, 2052:WPAD]
                        )
                    else:
                        nc.sync.dma_start(
                            out=xt[:], in_=xs[:, b * WPAD : (b + n) * WPAD]
                        )
                    in_b0 = b
                if b in out_starts:
                    out_n = out_starts[b]
                    ot = opool.tile([128, out_n * WO], MM_DT, tag="ot", name=f"ot{b}")
                    out_b0 = b
                xoff = (b - in_b0) * WPAD
                ooff = (b - out_b0) * WO
                for u in range(8):
                    ps = ppool.tile(
                        [128, 512], F32, tag="ps", name=f"ps_{b}_{u}"
                    )
                    for dx in range(3):
                        band = bt[:, 128 * dx : 128 * (dx + 1)]
                        s = xoff + 512 * u + dx
                        nc.tensor.matmul(
                            ps[:],
                            band,
                            xt[:, s : s + 512],
                            start=(dx == 0),
                            stop=(dx == 2),
                            skip_group_check=True,
                        )
                    dst = ot[:, ooff + 512 * u : ooff + 512 * u + 512]
                    if whi % 2 == 1:
                        nc.scalar.copy(dst, ps[:])
                    else:
                        nc.vector.tensor_copy(dst, ps[:])
                    whi += 1
                    if b == NBLK - 1 and u in (3, 7):
                        # tail: overlap the last write with the last copies
                        eng = nc.sync if u == 3 else nc.scalar
                        eng.dma_start(
                            out=ys[:, b * WO + (u - 3) * 512 : b * WO + (u + 1) * 512],
                            in_=ot[0:120, ooff + (u - 3) * 512 : ooff + (u + 1) * 512],
                        )
                if b == out_b0 + out_n - 1 and b != NBLK - 1:
                    eng = nc.sync if oci % 2 == 0 else nc.scalar
                    eng.dma_start(
                        out=ys[:, out_b0 * WO : (b + 1) * WO],
                        in_=ot[0:120, :],
                    )
                    oci += 1

    nc.compile()
    return nc


def _make_bands(kw: np.ndarray):
    import ml_dtypes

    bands = np.zeros((128, 3 * 128), dtype=np.float32)
    m = np.arange(YB)
    for dx in range(3):
        for co in range(C):
            for ci in range(C):
                for dy in range(3):
                    bands[32 * ci + m + dy, 128 * dx + 30 * co + m] = kw[
                        co, ci, dy, dx
                    ]
    return bands.astype(ml_dtypes.bfloat16)


def _prep_inputs(x: np.ndarray, kw: np.ndarray) -> list[dict]:
    import ml_dtypes

    bdt = ml_dtypes.bfloat16
    xpad = np.zeros((C, H + 2, W + 2), dtype=bdt)
    xpad[:, 1 : H + 1, 1 : W + 1] = x.astype(bdt)
    bands = _make_bands(kw)
    row_idx = np.empty((NBLK, 32), dtype=np.int64)
    for b in range(NBLK):
        row_idx[b] = _r0(b) + np.arange(32)
    maps = []
    for c in range(N_CORES):
        g = xpad[:, SH * c + row_idx, :]            # [C, NBLK, 32, WPAD]
        packed = np.ascontiguousarray(
            g.transpose(0, 2, 1, 3).reshape(128, NBLK * WPAD)
        )
        maps.append({"xs": packed, "bands": bands})
    return maps


def _unshard(results) -> np.ndarray:
    out = np.empty((C, H, W), dtype=np.float32)
    for c in range(N_CORES):
        ys_c = np.asarray(results[c]["ys"])          # [120, NBLK*WO] bf16
        t = ys_c.reshape(C, YB, NBLK, WO).transpose(0, 2, 1, 3)  # [co, b, m, w]
        oc = out[:, SH * c : SH * (c + 1), :]
        oc[:, : YB * (NBLK - 1)] = t[:, : NBLK - 1].reshape(C, YB * (NBLK - 1), WO)
        oc[:, YB * (NBLK - 1) :] = t[:, NBLK - 1, YB - 2 :]
    return out


def kernel(x: np.ndarray, kernel: np.ndarray) -> np.ndarray:
    x = np.asarray(x, dtype=np.float32)
    kw = np.asarray(kernel, dtype=np.float32)

    if "nc" not in _CACHE:
        _CACHE["nc"] = _build_program()
    nc = _CACHE["nc"]

    in_maps = _prep_inputs(x, kw)
    res = run_bass_kernel_spmd(nc, in_maps, list(range(N_CORES)))
    return _unshard(res.results)
